# revision 62
# baseline (speedup 1.0000x reference)
"""Multi-head attention (B=4, S=2048, D=1024, H=16) on 8 TRN2 NeuronCores.

Sharding (Megatron-style, per spec hint): data-parallel over batch (4) x
tensor-parallel over heads (2 groups of 8). Core c handles batch c//2,
head-group c%2. QKV projections column-sharded, output projection
row-sharded; the two partial outputs per batch are summed on the host
together with the output bias (host also upcasts the bf16 output).

Per-core kernel (one NeuronCore, 8 heads, 2048 tokens), ~337us in the
InstructionCostModel timeline (baseline was 377.5us):
  - ACT (exp) is the pacing engine: one [128, 1024] exp per score-tile
    pair (~2.08us per kt2 slot, 128 slots, ~267us busy). PE total is
    ~280us, so both engines must stream with few gaps.
  - Scores transposed ST[k, q] = K Q^T; softmax skips max-subtraction
    (logits ~N(0,1), safe for fp32 exp); 1/sqrt(dk) folded into ACT's
    scale; probabilities written bf16.
  - att@V is est-STATIONARY: av[q=128, 65] = est[k,q]^T @ v_aug[k,65]
    chained over 16 k-tiles. All 128 out partitions are used, so the
    matmul cost is 65 moving rows per k-tile -- half the est-moving
    form. The ones column of v_aug lands the softmax denominator Z in
    av[:, 64] per token partition, making normalize a per-partition
    reciprocal + tensor_scalar broadcast (no PE broadcast matmul).
  - Normalized [token, feature] 128x128 bf16 head-pair tiles transpose
    to feature-major oT via the DMA crossbar (dma_start_transpose),
    off the PE/DVE critical path; outproj consumes oT bf16-stationary
    against a bf16 wo.
  - The DMA unit runs ONE transfer at a time in the cost model, so the
    load emission order is the transfer schedule: wk/xk0/wq/xq0 first
    (wk/wq host-swizzled to contiguous 256KB fc-slices), then k-groups
    for the JIT kproj splices, then v, then later q-groups. wo rides
    the ACT hwdge queue into an x-pool buf that frees mid-kernel.
  - The tile scheduler list-schedules by readiness with emission index
    as priority: scores+exp emitted inside high_priority(500000),
    att@V units at 250000, so the score/exp stream preempts fill work
    whenever both are ready. Background PSUM (vproj/attV/outproj) is
    segregated from the kproj/qproj feed pool to prevent priority
    inversion through buffer WAR. Staging pools (o_tm, rz, osb) are
    4-5 deep so lagging DMA transposes/stores never back-pressure the
    DVE normalize that frees est tiles.
  - kproj chain (kg, fc) is spliced just before the first scores that
    need it; qproj for pair t+1 emits inside pair t; per-head-pair
    vproj units are ready-slot gated to the xv DMA schedule; att@V and
    outproj drain a FIFO in the score-loop slack. PE warmup/cooldown
    matmuls keep the clock at full p-state across idle edges.
"""

import sys

if "/opt/trn_rl_repo" not in sys.path:
    sys.path.insert(0, "/opt/trn_rl_repo")

import numpy as np

B, S, D = 4, 2048, 1024
H, DK = 16, 64
NCORES = 8
HC = H // 2            # heads per core
DC = HC * DK           # 512 local features per core
INV_SCALE = 1.0 / 8.0  # 1/sqrt(DK)
P = 128
NDCH = D // P          # 8 contraction chunks for projections
NFC = DC // P          # 4 local feature chunks (== head pairs)
NKT = S // P           # 16 key tiles
NQG = 4                # query groups
QG = S // NQG          # 512 queries per group
VW = DK + 1            # 65: v columns + ones column
NHP = HC // 2          # head pairs
NPAIR = NQG * NHP      # 16 (group, pair) units
import os
KNOB_PRI_FEED = int(os.environ.get('K_PRI_FEED', '0'))
KNOB_PRI_ATTV = int(os.environ.get('K_PRI_ATTV', '1'))
KNOB_POOLSPLIT = int(os.environ.get('K_POOLSPLIT', '1'))
KNOB_OPBATCH = int(os.environ.get('K_OPBATCH', '1'))
KNOB_PRI_SCORE = int(os.environ.get('K_PRI_SCORE', '1'))
EBUFS = int(os.environ.get('K_EBUFS', '26'))             # est tiles in flight (SBUF-limited)

# PE cost accounting for the fill queue (ns, cost-model units)
ROW_NS = 0.4167
SLOT_NS = 2076.0       # ACT cadence per kt2 slot (2 exps)
SCORE_NS = 4 * 512 * ROW_NS
CHAIN_NS = 8 * 512 * ROW_NS    # one 8-deep projection chain
AV_NS = 16 * VW * ROW_NS       # one att@V chain (16 matmuls, 65 rows)
OPROJ_NS = 4 * 512 * ROW_NS    # one outproj chain

_CACHE = {}


def _build():
    import concourse.bass as bass
    import concourse.bacc as bacc
    import concourse.tile as tile
    import concourse.mybir as mybir
    from concourse.bass import ts, ds

    f32 = mybir.dt.float32
    bf16 = mybir.dt.bfloat16
    AF = mybir.ActivationFunctionType
    ALU = mybir.AluOpType

    nc = bacc.Bacc("TRN2", target_bir_lowering=False, num_devices=NCORES)

    xqT = nc.dram_tensor("xqT", [D, S], bf16, kind="ExternalInput")
    xkT = nc.dram_tensor("xkT", [D, S], bf16, kind="ExternalInput")
    xvT = nc.dram_tensor("xvT", [D, S], bf16, kind="ExternalInput")
    # wq/wk pre-swizzled on host to [fc, p, dch, 128] so a single fc slice
    # is one contiguous-descriptor 256KB DMA (728ns on the serial DMA unit)
    wq = nc.dram_tensor("wq", [NFC, P, NDCH, P], bf16, kind="ExternalInput")
    wk = nc.dram_tensor("wk", [NFC, P, NDCH, P], bf16, kind="ExternalInput")
    wv = nc.dram_tensor("wv", [D, DC], bf16, kind="ExternalInput")
    wo = nc.dram_tensor("wo", [DC, D], bf16, kind="ExternalInput")
    bq = nc.dram_tensor("bq", [DC], f32, kind="ExternalInput")
    bk = nc.dram_tensor("bk", [DC], f32, kind="ExternalInput")
    bv = nc.dram_tensor("bv", [DC], f32, kind="ExternalInput")
    # bf16 output: halves store-DMA time and SBUF staging; host upcasts
    out = nc.dram_tensor("out", [S, D], bf16, kind="ExternalOutput")

    with tile.TileContext(nc) as tc:
        with (
            tc.tile_pool(name="persist", bufs=1) as persist,
            tc.tile_pool(name="wts", bufs=2) as wpool,
            tc.tile_pool(name="xin", bufs=9) as xpool,
            tc.tile_pool(name="qt", bufs=2) as qpool,
            tc.tile_pool(name="expst", bufs=EBUFS) as epool,
            tc.tile_pool(name="outt", bufs=2) as opool,
            tc.tile_pool(name="small", bufs=2) as spool,
            tc.tile_pool(name="osb", bufs=2) as osb_pool,
            tc.tile_pool(name="misc", bufs=2, space="PSUM") as pp,
            tc.tile_pool(name="st", bufs=2, space="PSUM") as st_pool,
            tc.tile_pool(name="av", bufs=2, space="PSUM") as avp,
        ):
            # ---- persistent SBUF tensors ----
            kT = persist.tile([P, NFC, S], bf16)          # 16KB/part
            v_aug = persist.tile([P, NKT, HC, VW], bf16)  # ~16.6KB/part
            bq_sb = persist.tile([P, NFC], f32)
            bk_sb = persist.tile([P, NFC], f32)
            bvb = persist.tile([P, DC], f32)              # bias_v broadcast

            # biases on the ACT hwdge queue (ACT idle until first exp) so
            # the SP queue stays clear for the critical x/w loads
            nc.scalar.dma_start(out=bq_sb, in_=bq.rearrange("(c p) -> p c", p=P))
            nc.scalar.dma_start(out=bk_sb, in_=bk.rearrange("(c p) -> p c", p=P))
            bv_ap = bv.ap()
            bvb_src = bass.AP(
                tensor=bv_ap.tensor, offset=bv_ap.offset, ap=[[0, P], *bv_ap.ap]
            )
            nc.scalar.dma_start(out=bvb, in_=bvb_src)
            # ones column for the Z trick (also doubles as PE warmup operand)
            ones_st = persist.tile([P, P], f32)
            nc.vector.memset(ones_st, 1.0)
            nc.vector.tensor_copy(
                out=v_aug[:, :, :, DK],
                in_=ones_st.rearrange("p (k h) -> p k h", k=NKT),
            )

            # ---- input loads (SP queue; the DMA unit runs ONE transfer at
            # a time, so emission order below is the transfer schedule) ----
            wk_sb = wpool.tile([P, NFC, NDCH, P], bf16, tag="w", name="w_k")
            wq_sb = wpool.tile([P, NFC, NDCH, P], bf16, tag="wq", name="w_q", bufs=1)
            xk_sbs, xq_sbs, xv_sbs = [], [], []

            def load_wfc(w_dram, w_sb, fc):
                nc.sync.dma_start(out=w_sb[:, fc], in_=w_dram[fc])

            def load_x(xT_dram, g, name, split=1):
                x_sb = xpool.tile([P, NDCH, QG], bf16, tag="x", name=name, bufs=8)
                xr = xT_dram.rearrange("(c p) t -> p c t", p=P)[:, :, ts(g, QG)]
                hh = NDCH // split
                for i in range(split):
                    nc.sync.dma_start(
                        out=x_sb[:, i * hh : (i + 1) * hh, :],
                        in_=xr[:, i * hh : (i + 1) * hh, :],
                    )
                return x_sb

            # pre-allocate x tiles in buf-priority order (9 bufs; the last
            # three reuse bufs of early-freed tiles), then issue DMAs in
            # transfer-schedule order on the serial DMA unit
            def xtile(name):
                return xpool.tile([P, NDCH, QG], bf16, tag="x", name=name, bufs=9)

            xk_sbs = [xtile(f"x_k_{g}") for g in range(NQG)]
            xq_sbs = [xtile("x_q_0")]
            xv_sbs = [xtile(f"x_v_{g}") for g in range(NQG)]
            xq_sbs += [xtile(f"x_q_{g}") for g in range(1, NQG)]
            # wo shares the 8KB x-slot pool: allocated 13th, it reuses
            # the buf of an x tile freed by the kproj/qproj prelude
            wo_sb = xpool.tile([P, NFC, D], bf16, tag="x", name="wo_sb", bufs=9)
            wv_sb = wpool.tile([P, NDCH, DC], bf16, tag="w", name="w_v")

            def load_x(x_sb, xT_dram, g, split=1):
                xr = xT_dram.rearrange("(c p) t -> p c t", p=P)[:, :, ts(g, QG)]
                hh = NDCH // split
                for i in range(split):
                    nc.sync.dma_start(
                        out=x_sb[:, i * hh : (i + 1) * hh, :],
                        in_=xr[:, i * hh : (i + 1) * hh, :],
                    )

            # critical path to the first exp: wk fc0, xk0, wq fc0, xq0
            load_wfc(wk, wk_sb, 0)
            load_x(xk_sbs[0], xkT, 0, split=2)
            load_wfc(wq, wq_sb, 0)
            load_x(xq_sbs[0], xqT, 0, split=2)
            # deadline-ordered remainder: k-groups for the JIT kproj
            # splices, v inputs early (attV est-pool relief), then the rest
            load_x(xk_sbs[1], xkT, 1)
            load_wfc(wq, wq_sb, 1)
            load_x(xk_sbs[2], xkT, 2)
            nc.sync.dma_start(out=wv_sb, in_=wv.rearrange("(c p) f -> p c f", p=P))
            load_x(xk_sbs[3], xkT, 3)
            load_wfc(wk, wk_sb, 1)
            load_x(xv_sbs[0], xvT, 0)
            load_wfc(wq, wq_sb, 2)
            load_x(xv_sbs[1], xvT, 1)
            load_wfc(wk, wk_sb, 2)
            load_x(xv_sbs[2], xvT, 2)
            load_wfc(wq, wq_sb, 3)
            load_x(xv_sbs[3], xvT, 3)
            load_wfc(wk, wk_sb, 3)
            load_x(xq_sbs[1], xqT, 1)
            # wo on the ACT hwdge queue: its buf frees only at ~56us (WAR on
            # an early x tile) and would block every transpose behind it on SP
            nc.scalar.dma_start(out=wo_sb, in_=wo.rearrange("(c p) e -> p c e", p=P))
            load_x(xq_sbs[2], xqT, 2)
            load_x(xq_sbs[3], xqT, 3)

            # ---- emission helpers (per-engine program order == emission
            # order; semaphores enforce cross-engine deps) ----
            # scheduler priorities: score/exp feed + their kproj/qproj
            # gates highest, attV (est-pool relief) mid, vproj/outproj base
            PRI_FEED = 500000 if KNOB_PRI_FEED else 0
            PRI_ATTV = 250000 if KNOB_PRI_ATTV else 0

            def kproj_chain(g, fc):
                with tc.high_priority(offset=PRI_FEED):
                    ps = pp.tile([P, QG], f32, tag="pp", name=f"pk_{g}_{fc}")
                    for dch in range(NDCH):
                        nc.tensor.matmul(
                            ps, wk_sb[:, fc, dch, :], xk_sbs[g][:, dch, :],
                            start=(dch == 0), stop=(dch == NDCH - 1),
                        )
                    nc.vector.tensor_scalar(
                        out=kT[:, fc, ts(g, QG)], in0=ps,
                        scalar1=bk_sb[:, fc : fc + 1], scalar2=None, op0=ALU.add,
                    )

            def qproj_chain(qT, g, fc):
                with tc.high_priority(offset=PRI_FEED):
                    ps = pp.tile([P, QG], f32, tag="pp", name=f"pq_{g}_{fc}")
                    for dch in range(NDCH):
                        nc.tensor.matmul(
                            ps, wq_sb[:, fc, dch, :], xq_sbs[g][:, dch, :],
                            start=(dch == 0), stop=(dch == NDCH - 1),
                        )
                    nc.vector.tensor_scalar(
                        out=qT[:, fc, :], in0=ps,
                        scalar1=bq_sb[:, fc : fc + 1], scalar2=None, op0=ALU.add,
                    )

            def vproj_unit(kt, hp2):
                # one k-tile x one head-pair (128 v features): fine-grained
                # so attV(pair hp2) unblocks without waiting for all of v;
                gg, tt = divmod(kt, QG // P)
                _bgp, _bgt = (avp, "av") if KNOB_POOLSPLIT else (pp, "pp")
                ps = _bgp.tile([P, P], f32, tag=_bgt, name=f"pv_{kt}_{hp2}")
                for dch in range(NDCH):
                    nc.tensor.matmul(
                        ps, xv_sbs[gg][:, dch, ts(tt, P)],
                        wv_sb[:, dch, ts(hp2, P)],
                        start=(dch == 0), stop=(dch == NDCH - 1),
                    )
                nc.vector.tensor_add(
                    out=v_aug[:, kt, 2 * hp2 : 2 * hp2 + 2, 0:DK],
                    in0=ps.rearrange("p (h d) -> p h d", h=2),
                    in1=bvb.rearrange("p (h d) -> p h d", h=HC)[
                        :, 2 * hp2 : 2 * hp2 + 2, :
                    ],
                )

            def attv_unit(t, qt, ests, oT):
                """One (pair, q-tile): both heads' est-stationary att@V
                chains, normalize, and the DMA-crossbar transpose into oT."""
                g, hp = divmod(t, NHP)
                qeng = nc.scalar if t == NPAIR - 1 else nc.sync
                with tc.high_priority(offset=PRI_ATTV):
                    o_tm = spool.tile([P, P], bf16, tag="otm", name=f"otm_{t}_{qt}", bufs=4)
                    rz = spool.tile([P, 2], f32, tag="rz", name=f"rz_{t}_{qt}", bufs=4)
                    for hi in range(2):
                        h = 2 * hp + hi
                        av = avp.tile([P, VW], f32, tag="av", name=f"av_{t}_{qt}_{hi}")
                        for kt in range(NKT):
                            nc.tensor.matmul(
                                av,
                                ests[hi][kt // 2][:, kt % 2, ts(qt, P)],
                                v_aug[:, kt, h, :],
                                start=(kt == 0), stop=(kt == NKT - 1),
                            )
                        nc.vector.reciprocal(
                            out=rz[:, hi : hi + 1], in_=av[:, DK : DK + 1]
                        )
                        nc.vector.tensor_scalar(
                            out=o_tm[:, hi * DK : (hi + 1) * DK], in0=av[:, 0:DK],
                            scalar1=rz[:, hi : hi + 1], scalar2=None, op0=ALU.mult,
                        )
                    qeng.dma_start_transpose(
                        out=oT[:, hp, ts(qt, P)], in_=o_tm
                    )

            def outproj_unit(oT, g, tt, eg):
                _bgp, _bgt = (avp, "av") if KNOB_POOLSPLIT else (pp, "pp")
                ps = _bgp.tile([P, DC], f32, tag=_bgt, name=f"po_{g}_{tt}_{eg}")
                for fc in range(NFC):
                    nc.tensor.matmul(
                        ps, oT[:, fc, ts(tt, P)], wo_sb[:, fc, ts(eg, DC)],
                        start=(fc == 0), stop=(fc == NFC - 1),
                    )
                o_sb = osb_pool.tile([P, DC], bf16, tag="osb", name=f"ob_{g}_{tt}_{eg}", bufs=5)
                nc.vector.tensor_copy(out=o_sb, in_=ps)
                qeng = nc.scalar if g == NQG - 1 else nc.sync
                qeng.dma_start(
                    out=out[ds(g * QG + tt * P, P), ts(eg, DC)], in_=o_sb
                )

            # ---- fill queue: deferrable PE work drained in score slack.
            # Items carry a ready_slot gate: PE is in-order, so popping an
            # item whose input DMA hasn't landed would stall the whole
            # stream. FIFO order is preserved (head-not-ready stops drain).
            fillq = []           # FIFO of (cost_ns, ready_slot, fn)
            state = {"budget": 0.0, "slot": 0}

            def drain(extra_budget):
                # greedy modulo ready_slot gates: the tile scheduler
                # list-schedules by readiness with emission index as
                # priority, so early emission just sets priority
                while fillq and fillq[0][1] <= state["slot"]:
                    cost, _, fn = fillq.pop(0)
                    fn()

            # ---- PE warmup during the initial DMA wait: fine-grained
            # matmuls keep the clock ramped without over-running the slot
            # where real inputs land ----
            for i in range(20):
                ps = pp.tile([P, P], f32, tag="pp", name=f"warm_{i}")
                nc.tensor.matmul(ps, ones_st, ones_st, start=True, stop=True)

            # ---- prelude: first chains for pair (0, 0) ----
            qts = {0: qpool.tile([P, NFC, QG], bf16, tag="qT", name="qT_0")}
            kproj_chain(0, 0)
            qproj_chain(qts[0], 0, 0)

            # vproj ready slots matched to the xv transfer schedule
            VREADY = (9, 11, 12, 14)

            def push_vproj(hp2):
                for kt in range(NKT):
                    fillq.append(
                        (CHAIN_NS / 4.0, VREADY[kt // 4],
                         lambda kt=kt, hp2=hp2: vproj_unit(kt, hp2))
                    )

            push_vproj(0)

            ests_by_pair = {}
            attv_emitted = [0] * NPAIR   # qt count emitted per pair
            oT_by_g = {}

            def push_attv(t):
                g = t // NHP
                if g not in oT_by_g:
                    oT_by_g[g] = opool.tile(
                        [P, NFC, QG], bf16, tag="oT", name=f"oT_{g}"
                    )
                oT = oT_by_g[g]
                ests = ests_by_pair[t]
                g2, hp2 = divmod(t, NHP)
                if KNOB_OPBATCH:
                    for qt in range(NQG):
                        fillq.append(
                            (AV_NS, 0,
                             lambda t=t, qt=qt, ests=ests, oT=oT:
                                 attv_unit(t, qt, ests, oT))
                        )
                    if hp2 == NHP - 1:
                        for tt in range(NQG):
                            for eg in range(2):
                                fillq.append(
                                    (OPROJ_NS, 0,
                                     lambda g2=g2, tt=tt, eg=eg:
                                         outproj_unit(oT_by_g[g2], g2, tt, eg))
                                )
                else:
                    for qt in range(NQG):
                        def unit(t=t, qt=qt, ests=ests, oT=oT, g2=g2, hp2=hp2):
                            attv_unit(t, qt, ests, oT)
                            if hp2 == NHP - 1:
                                for eg in range(2):
                                    outproj_unit(oT_by_g[g2], g2, qt, eg)
                        fillq.append((AV_NS + (2 * OPROJ_NS if hp2 == NHP - 1 else 0), 0, unit))

            # ---- main loop: 16 pairs x 8 kt2 slots ----
            for t in range(NPAIR):
                g, hp = divmod(t, NHP)
                qT = qts[g]
                ha, hb = 2 * hp, 2 * hp + 1
                ests = {0: [], 1: []}
                ests_by_pair[t] = ests
                for kt2 in range(NKT // 2):
                    state["slot"] = t * 8 + kt2
                    # mandatory JIT splices
                    spent = 0.0
                    if g == 0 and kt2 in (1, 3, 5):
                        kproj_chain((kt2 + 1) // 2, hp)
                        spent += CHAIN_NS
                    if g == 0 and kt2 == 6 and hp + 1 < NFC:
                        kproj_chain(0, hp + 1)
                        spent += CHAIN_NS
                    if kt2 == 2 and t + 1 < NPAIR:
                        g1, fc1 = divmod(t + 1, NHP)
                        if g1 not in qts:
                            qts[g1] = qpool.tile(
                                [P, NFC, QG], bf16, tag="qT", name=f"qT_{g1}"
                            )
                        qproj_chain(qts[g1], g1, fc1)
                        spent += CHAIN_NS
                    # fill-queue drain for the rest of this slot's slack
                    drain(SLOT_NS - SCORE_NS - spent)
                    # scores for kt = 2*kt2, 2*kt2+1 (both heads) + exp, at
                    # high scheduler priority: the list scheduler then only
                    # runs fill work when the score/exp stream is blocked
                    with tc.high_priority(offset=500000 if KNOB_PRI_SCORE else 0):
                        sts = {
                            hi: st_pool.tile(
                                [P, 2, QG], f32, tag="st", name=f"st_{t}_{hi}_{kt2}"
                            )
                            for hi in range(2)
                        }
                        for kk in range(2):
                            kt = 2 * kt2 + kk
                            for hi in range(2):
                                r0 = hi * DK
                                nc.tensor.matmul(
                                    sts[hi][:, kk, :],
                                    kT[r0 : r0 + DK, hp, ts(kt, P)],
                                    qT[r0 : r0 + DK, hp, :],
                                    start=True, stop=True, tile_position=(r0, 0),
                                )
                        for hi in range(2):
                            e = epool.tile(
                                [P, 2, QG], bf16, tag="est", name=f"est_{t}_{hi}_{kt2}"
                            )
                            ests[hi].append(e)
                            nc.scalar.activation(
                                out=e, in_=sts[hi], func=AF.Exp, scale=INV_SCALE
                            )
                # this pair's est tiles are complete -> queue its att@V;
                # stage the next head-pair's vproj ahead of the next attV
                push_attv(t)
                if t < NHP - 1:
                    push_vproj(t + 1)

            # ---- tail: drain everything left ----
            state["slot"] = 10**9
            drain(1e9)
            for i in range(16):
                ps = pp.tile([P, P], f32, tag="pp", name=f"cool_{i}")
                nc.tensor.matmul(ps, ones_st, ones_st, start=True, stop=True)

    nc.compile()
    return nc


def _get_nc(debug=False):
    if "nc" not in _CACHE:
        _CACHE["nc"] = _build()
    return _CACHE["nc"]


def _bf16(a):
    import ml_dtypes

    return np.ascontiguousarray(a, dtype=np.float32).astype(ml_dtypes.bfloat16)


def _make_in_maps(inputs):
    q = np.asarray(inputs["query"], dtype=np.float32)
    k = np.asarray(inputs["key"], dtype=np.float32)
    v = np.asarray(inputs["value"], dtype=np.float32)
    wq = np.asarray(inputs["wq"], dtype=np.float32)
    wk = np.asarray(inputs["wk"], dtype=np.float32)
    wv = np.asarray(inputs["wv"], dtype=np.float32)
    wo = np.asarray(inputs["wo"], dtype=np.float32)
    bq = np.asarray(inputs["bq"], dtype=np.float32)
    bk = np.asarray(inputs["bk"], dtype=np.float32)
    bv = np.asarray(inputs["bv"], dtype=np.float32)

    def _wsw(w):
        # [D, DC] -> [NFC, P, NDCH, P]: fc-major so one fc slice is a
        # single contiguous 256KB DMA
        return _bf16(
            np.ascontiguousarray(
                w.reshape(NDCH, P, NFC, P).transpose(2, 1, 0, 3)
            )
        )

    xT = [(_bf16(q[b].T), _bf16(k[b].T), _bf16(v[b].T)) for b in range(B)]
    in_maps = []
    for c in range(NCORES):
        b, g = divmod(c, 2)
        sl = slice(g * DC, (g + 1) * DC)
        in_maps.append(
            {
                "xqT": xT[b][0],
                "xkT": xT[b][1],
                "xvT": xT[b][2],
                "wq": _wsw(wq[:, sl]),
                "wk": _wsw(wk[:, sl]),
                "wv": _bf16(wv[:, sl]),
                "wo": _bf16(wo[sl, :]),
                "bq": np.ascontiguousarray(bq[sl]),
                "bk": np.ascontiguousarray(bk[sl]),
                "bv": np.ascontiguousarray(bv[sl]),
            }
        )
    return in_maps


def run(inputs, **kwargs):
    """Run the kernel; returns (full_output, BassKernelResults)."""
    from concourse.bass_utils import run_bass_kernel_spmd

    kwargs.pop("debug", None)
    nc = _get_nc()
    in_maps = _make_in_maps(inputs)
    res = run_bass_kernel_spmd(nc, in_maps, core_ids=list(range(NCORES)), **kwargs)
    bo = np.asarray(inputs["bo"], dtype=np.float32)
    final = np.empty((B, S, D), np.float32)
    for b in range(B):
        final[b] = (
            np.asarray(res.results[2 * b]["out"], dtype=np.float32)
            + np.asarray(res.results[2 * b + 1]["out"], dtype=np.float32)
            + bo
        )
    return final, res


def kernel(**inputs):
    return run(inputs)[0]


# revision 65
# speedup vs baseline: 1.0042x; 1.0042x over previous
"""Multi-head attention (B=4, S=2048, D=1024, H=16) on 8 TRN2 NeuronCores.

Sharding (Megatron-style, per spec hint): data-parallel over batch (4) x
tensor-parallel over heads (2 groups of 8). Core c handles batch c//2,
head-group c%2. QKV projections column-sharded, output projection
row-sharded; the two partial outputs per batch are summed on the host
together with the output bias (host also upcasts the bf16 output).

Per-core kernel (one NeuronCore, 8 heads, 2048 tokens), ~337us in the
InstructionCostModel timeline (baseline was 377.5us):
  - ACT (exp) is the pacing engine: one [128, 1024] exp per score-tile
    pair (~2.08us per kt2 slot, 128 slots, ~267us busy). PE total is
    ~280us, so both engines must stream with few gaps.
  - Scores transposed ST[k, q] = K Q^T; softmax skips max-subtraction
    (logits ~N(0,1), safe for fp32 exp); 1/sqrt(dk) folded into ACT's
    scale; probabilities written bf16.
  - att@V is est-STATIONARY: av[q=128, 65] = est[k,q]^T @ v_aug[k,65]
    chained over 16 k-tiles. All 128 out partitions are used, so the
    matmul cost is 65 moving rows per k-tile -- half the est-moving
    form. The ones column of v_aug lands the softmax denominator Z in
    av[:, 64] per token partition, making normalize a per-partition
    reciprocal + tensor_scalar broadcast (no PE broadcast matmul).
  - Normalized [token, feature] 128x128 bf16 head-pair tiles transpose
    to feature-major oT via the DMA crossbar (dma_start_transpose),
    off the PE/DVE critical path; outproj consumes oT bf16-stationary
    against a bf16 wo.
  - The DMA unit runs ONE transfer at a time in the cost model, so the
    load emission order is the transfer schedule: wk/xk0/wq/xq0 first
    (wk/wq host-swizzled to contiguous 256KB fc-slices), then k-groups
    for the JIT kproj splices, then v, then later q-groups. wo rides
    the ACT hwdge queue into an x-pool buf that frees mid-kernel.
  - The tile scheduler list-schedules by readiness with emission index
    as priority: scores+exp emitted inside high_priority(500000),
    att@V units at 250000, so the score/exp stream preempts fill work
    whenever both are ready. Background PSUM (vproj/attV/outproj) is
    segregated from the kproj/qproj feed pool to prevent priority
    inversion through buffer WAR. Staging pools (o_tm, rz, osb) are
    4-5 deep so lagging DMA transposes/stores never back-pressure the
    DVE normalize that frees est tiles.
  - kproj chain (kg, fc) is spliced just before the first scores that
    need it; qproj for pair t+1 emits inside pair t; per-head-pair
    vproj units are ready-slot gated to the xv DMA schedule; att@V and
    outproj drain a FIFO in the score-loop slack. PE warmup/cooldown
    matmuls keep the clock at full p-state across idle edges.
"""

import sys

if "/opt/trn_rl_repo" not in sys.path:
    sys.path.insert(0, "/opt/trn_rl_repo")

import numpy as np

B, S, D = 4, 2048, 1024
H, DK = 16, 64
NCORES = 8
HC = H // 2            # heads per core
DC = HC * DK           # 512 local features per core
INV_SCALE = 1.0 / 8.0  # 1/sqrt(DK)
P = 128
NDCH = D // P          # 8 contraction chunks for projections
NFC = DC // P          # 4 local feature chunks (== head pairs)
NKT = S // P           # 16 key tiles
NQG = 4                # query groups
QG = S // NQG          # 512 queries per group
VW = DK + 1            # 65: v columns + ones column
NHP = HC // 2          # head pairs
NPAIR = NQG * NHP      # 16 (group, pair) units
import os
KNOB_PRI_FEED = int(os.environ.get('K_PRI_FEED', '0'))
KNOB_PRI_ATTV = int(os.environ.get('K_PRI_ATTV', '1'))
KNOB_POOLSPLIT = int(os.environ.get('K_POOLSPLIT', '1'))
KNOB_OPBATCH = int(os.environ.get('K_OPBATCH', '1'))
KNOB_PRI_SCORE = int(os.environ.get('K_PRI_SCORE', '1'))
EBUFS = int(os.environ.get('K_EBUFS', '26'))             # est tiles in flight (SBUF-limited)

# PE cost accounting for the fill queue (ns, cost-model units)
ROW_NS = 0.4167
SLOT_NS = 2076.0       # ACT cadence per kt2 slot (2 exps)
SCORE_NS = 4 * 512 * ROW_NS
CHAIN_NS = 8 * 512 * ROW_NS    # one 8-deep projection chain
AV_NS = 16 * VW * ROW_NS       # one att@V chain (16 matmuls, 65 rows)
OPROJ_NS = 4 * 512 * ROW_NS    # one outproj chain

_CACHE = {}


def _build():
    import concourse.bass as bass
    import concourse.bacc as bacc
    import concourse.tile as tile
    import concourse.mybir as mybir
    from concourse.bass import ts, ds

    f32 = mybir.dt.float32
    bf16 = mybir.dt.bfloat16
    AF = mybir.ActivationFunctionType
    ALU = mybir.AluOpType

    nc = bacc.Bacc("TRN2", target_bir_lowering=False, num_devices=NCORES)

    xqT = nc.dram_tensor("xqT", [D, S], bf16, kind="ExternalInput")
    xkT = nc.dram_tensor("xkT", [D, S], bf16, kind="ExternalInput")
    xvT = nc.dram_tensor("xvT", [D, S], bf16, kind="ExternalInput")
    # wq/wk pre-swizzled on host to [fc, p, dch, 128] so a single fc slice
    # is one contiguous-descriptor 256KB DMA (728ns on the serial DMA unit)
    wq = nc.dram_tensor("wq", [NFC, P, NDCH, P], bf16, kind="ExternalInput")
    wk = nc.dram_tensor("wk", [NFC, P, NDCH, P], bf16, kind="ExternalInput")
    wv = nc.dram_tensor("wv", [D, DC], bf16, kind="ExternalInput")
    wo = nc.dram_tensor("wo", [DC, D], bf16, kind="ExternalInput")
    bq = nc.dram_tensor("bq", [DC], f32, kind="ExternalInput")
    bk = nc.dram_tensor("bk", [DC], f32, kind="ExternalInput")
    bv = nc.dram_tensor("bv", [DC], f32, kind="ExternalInput")
    # bf16 output: halves store-DMA time and SBUF staging; host upcasts
    out = nc.dram_tensor("out", [S, D], bf16, kind="ExternalOutput")

    with tile.TileContext(nc) as tc:
        with (
            tc.tile_pool(name="persist", bufs=1) as persist,
            tc.tile_pool(name="wts", bufs=2) as wpool,
            tc.tile_pool(name="xin", bufs=9) as xpool,
            tc.tile_pool(name="qt", bufs=2) as qpool,
            tc.tile_pool(name="expst", bufs=EBUFS) as epool,
            tc.tile_pool(name="outt", bufs=2) as opool,
            tc.tile_pool(name="small", bufs=2) as spool,
            tc.tile_pool(name="osb", bufs=2) as osb_pool,
            tc.tile_pool(name="misc", bufs=2, space="PSUM") as pp,
            tc.tile_pool(name="st", bufs=2, space="PSUM") as st_pool,
            tc.tile_pool(name="av", bufs=2, space="PSUM") as avp,
        ):
            # ---- persistent SBUF tensors ----
            kT = persist.tile([P, NFC, S], bf16)          # 16KB/part
            v_aug = persist.tile([P, NKT, HC, VW], bf16)  # ~16.6KB/part
            bq_sb = persist.tile([P, NFC], f32)
            bk_sb = persist.tile([P, NFC], f32)
            bvb = persist.tile([P, DC], f32)              # bias_v broadcast

            # biases on the ACT hwdge queue (ACT idle until first exp) so
            # the SP queue stays clear for the critical x/w loads
            nc.scalar.dma_start(out=bq_sb, in_=bq.rearrange("(c p) -> p c", p=P))
            nc.scalar.dma_start(out=bk_sb, in_=bk.rearrange("(c p) -> p c", p=P))
            bv_ap = bv.ap()
            bvb_src = bass.AP(
                tensor=bv_ap.tensor, offset=bv_ap.offset, ap=[[0, P], *bv_ap.ap]
            )
            nc.scalar.dma_start(out=bvb, in_=bvb_src)
            # ones column for the Z trick (also doubles as PE warmup operand)
            ones_st = persist.tile([P, P], f32)
            nc.vector.memset(ones_st, 1.0)
            nc.vector.tensor_copy(
                out=v_aug[:, :, :, DK],
                in_=ones_st.rearrange("p (k h) -> p k h", k=NKT),
            )

            # ---- input loads (SP queue; the DMA unit runs ONE transfer at
            # a time, so emission order below is the transfer schedule) ----
            wk_sb = wpool.tile([P, NFC, NDCH, P], bf16, tag="w", name="w_k")
            wq_sb = wpool.tile([P, NFC, NDCH, P], bf16, tag="wq", name="w_q", bufs=1)
            xk_sbs, xq_sbs, xv_sbs = [], [], []

            def load_wfc(w_dram, w_sb, fc):
                nc.sync.dma_start(out=w_sb[:, fc], in_=w_dram[fc])

            def load_x(xT_dram, g, name, split=1):
                x_sb = xpool.tile([P, NDCH, QG], bf16, tag="x", name=name, bufs=8)
                xr = xT_dram.rearrange("(c p) t -> p c t", p=P)[:, :, ts(g, QG)]
                hh = NDCH // split
                for i in range(split):
                    nc.sync.dma_start(
                        out=x_sb[:, i * hh : (i + 1) * hh, :],
                        in_=xr[:, i * hh : (i + 1) * hh, :],
                    )
                return x_sb

            # pre-allocate x tiles in buf-priority order (9 bufs; the last
            # three reuse bufs of early-freed tiles), then issue DMAs in
            # transfer-schedule order on the serial DMA unit
            def xtile(name):
                return xpool.tile([P, NDCH, QG], bf16, tag="x", name=name, bufs=9)

            xk_sbs = [xtile(f"x_k_{g}") for g in range(NQG)]
            xq_sbs = [xtile("x_q_0")]
            xv_sbs = [xtile(f"x_v_{g}") for g in range(NQG)]
            xq_sbs += [xtile(f"x_q_{g}") for g in range(1, NQG)]
            # wo shares the 8KB x-slot pool: allocated 13th, it reuses
            # the buf of an x tile freed by the kproj/qproj prelude
            wo_sb = xpool.tile([P, NFC, D], bf16, tag="x", name="wo_sb", bufs=9)
            wv_sb = wpool.tile([P, NDCH, DC], bf16, tag="w", name="w_v")

            def load_x(x_sb, xT_dram, g, split=1):
                xr = xT_dram.rearrange("(c p) t -> p c t", p=P)[:, :, ts(g, QG)]
                hh = NDCH // split
                for i in range(split):
                    nc.sync.dma_start(
                        out=x_sb[:, i * hh : (i + 1) * hh, :],
                        in_=xr[:, i * hh : (i + 1) * hh, :],
                    )

            # critical path to the first exp: wk fc0, xk0, wq fc0, xq0
            load_wfc(wk, wk_sb, 0)
            load_x(xk_sbs[0], xkT, 0, split=2)
            load_wfc(wq, wq_sb, 0)
            load_x(xq_sbs[0], xqT, 0, split=2)
            # deadline-ordered remainder: k-groups for the JIT kproj
            # splices, v inputs early (attV est-pool relief), then the rest
            load_x(xk_sbs[1], xkT, 1)
            load_wfc(wq, wq_sb, 1)
            load_x(xk_sbs[2], xkT, 2)
            nc.sync.dma_start(out=wv_sb, in_=wv.rearrange("(c p) f -> p c f", p=P))
            load_x(xk_sbs[3], xkT, 3)
            load_wfc(wk, wk_sb, 1)
            load_x(xv_sbs[0], xvT, 0)
            load_wfc(wq, wq_sb, 2)
            load_x(xv_sbs[1], xvT, 1)
            load_wfc(wk, wk_sb, 2)
            load_x(xv_sbs[2], xvT, 2)
            load_wfc(wq, wq_sb, 3)
            load_x(xv_sbs[3], xvT, 3)
            load_wfc(wk, wk_sb, 3)
            load_x(xq_sbs[1], xqT, 1)
            # wo on the ACT hwdge queue: its buf frees only at ~56us (WAR on
            # an early x tile) and would block every transpose behind it on SP
            nc.scalar.dma_start(out=wo_sb, in_=wo.rearrange("(c p) e -> p c e", p=P))
            load_x(xq_sbs[2], xqT, 2)
            load_x(xq_sbs[3], xqT, 3)

            # ---- emission helpers (per-engine program order == emission
            # order; semaphores enforce cross-engine deps) ----
            # scheduler priorities: score/exp feed + their kproj/qproj
            # gates highest, attV (est-pool relief) mid, vproj/outproj base
            PRI_FEED = 500000 if KNOB_PRI_FEED else 0
            PRI_ATTV = 250000 if KNOB_PRI_ATTV else 0

            def kproj_chain(g, fc):
                with tc.high_priority(offset=PRI_FEED):
                    ps = pp.tile([P, QG], f32, tag="pp", name=f"pk_{g}_{fc}")
                    for dch in range(NDCH):
                        nc.tensor.matmul(
                            ps, wk_sb[:, fc, dch, :], xk_sbs[g][:, dch, :],
                            start=(dch == 0), stop=(dch == NDCH - 1),
                        )
                    nc.vector.tensor_scalar(
                        out=kT[:, fc, ts(g, QG)], in0=ps,
                        scalar1=bk_sb[:, fc : fc + 1], scalar2=None, op0=ALU.add,
                    )

            def qproj_chain(qT, g, fc):
                with tc.high_priority(offset=PRI_FEED):
                    ps = pp.tile([P, QG], f32, tag="pp", name=f"pq_{g}_{fc}")
                    for dch in range(NDCH):
                        nc.tensor.matmul(
                            ps, wq_sb[:, fc, dch, :], xq_sbs[g][:, dch, :],
                            start=(dch == 0), stop=(dch == NDCH - 1),
                        )
                    nc.vector.tensor_scalar(
                        out=qT[:, fc, :], in0=ps,
                        scalar1=bq_sb[:, fc : fc + 1], scalar2=None, op0=ALU.add,
                    )

            def vproj_unit(kt, hp2):
                # one k-tile x one head-pair (128 v features): fine-grained
                # so attV(pair hp2) unblocks without waiting for all of v;
                gg, tt = divmod(kt, QG // P)
                _bgp, _bgt = (avp, "av") if KNOB_POOLSPLIT else (pp, "pp")
                ps = _bgp.tile([P, P], f32, tag=_bgt, name=f"pv_{kt}_{hp2}")
                for dch in range(NDCH):
                    nc.tensor.matmul(
                        ps, xv_sbs[gg][:, dch, ts(tt, P)],
                        wv_sb[:, dch, ts(hp2, P)],
                        start=(dch == 0), stop=(dch == NDCH - 1),
                    )
                nc.vector.tensor_add(
                    out=v_aug[:, kt, 2 * hp2 : 2 * hp2 + 2, 0:DK],
                    in0=ps.rearrange("p (h d) -> p h d", h=2),
                    in1=bvb.rearrange("p (h d) -> p h d", h=HC)[
                        :, 2 * hp2 : 2 * hp2 + 2, :
                    ],
                )

            def attv_unit(t, qt, ests, oT):
                """One (pair, q-tile): both heads' est-stationary att@V
                chains, normalize, and the DMA-crossbar transpose into oT."""
                g, hp = divmod(t, NHP)
                qeng = nc.scalar if t == NPAIR - 1 else nc.sync
                with tc.high_priority(offset=PRI_ATTV):
                    o_tm = spool.tile([P, P], bf16, tag="otm", name=f"otm_{t}_{qt}", bufs=4)
                    rz = spool.tile([P, 2], f32, tag="rz", name=f"rz_{t}_{qt}", bufs=4)
                    for hi in range(2):
                        h = 2 * hp + hi
                        av = avp.tile([P, VW], f32, tag="av", name=f"av_{t}_{qt}_{hi}")
                        for kt in range(NKT):
                            nc.tensor.matmul(
                                av,
                                ests[hi][kt // 2][:, kt % 2, ts(qt, P)],
                                v_aug[:, kt, h, :],
                                start=(kt == 0), stop=(kt == NKT - 1),
                            )
                        nc.vector.reciprocal(
                            out=rz[:, hi : hi + 1], in_=av[:, DK : DK + 1]
                        )
                        nc.vector.tensor_scalar(
                            out=o_tm[:, hi * DK : (hi + 1) * DK], in0=av[:, 0:DK],
                            scalar1=rz[:, hi : hi + 1], scalar2=None, op0=ALU.mult,
                        )
                    qeng.dma_start_transpose(
                        out=oT[:, hp, ts(qt, P)], in_=o_tm
                    )

            def outproj_unit(oT, g, tt, eg):
                _bgp, _bgt = (avp, "av") if KNOB_POOLSPLIT else (pp, "pp")
                ps = _bgp.tile([P, DC], f32, tag=_bgt, name=f"po_{g}_{tt}_{eg}")
                for fc in range(NFC):
                    nc.tensor.matmul(
                        ps, oT[:, fc, ts(tt, P)], wo_sb[:, fc, ts(eg, DC)],
                        start=(fc == 0), stop=(fc == NFC - 1),
                    )
                o_sb = osb_pool.tile([P, DC], bf16, tag="osb", name=f"ob_{g}_{tt}_{eg}", bufs=5)
                nc.vector.tensor_copy(out=o_sb, in_=ps)
                qeng = nc.scalar if g == NQG - 1 else nc.sync
                qeng.dma_start(
                    out=out[ds(g * QG + tt * P, P), ts(eg, DC)], in_=o_sb
                )

            # ---- fill queue: deferrable PE work drained in score slack.
            # Items carry a ready_slot gate: PE is in-order, so popping an
            # item whose input DMA hasn't landed would stall the whole
            # stream. FIFO order is preserved (head-not-ready stops drain).
            fillq = []           # FIFO of (cost_ns, ready_slot, fn)
            state = {"budget": 0.0, "slot": 0}

            def drain(extra_budget):
                # greedy modulo ready_slot gates: the tile scheduler
                # list-schedules by readiness with emission index as
                # priority, so early emission just sets priority
                while fillq and fillq[0][1] <= state["slot"]:
                    cost, _, fn = fillq.pop(0)
                    fn()

            # ---- PE warmup during the initial DMA wait: fine-grained
            # matmuls keep the clock ramped without over-running the slot
            # where real inputs land ----
            for i in range(20):
                ps = pp.tile([P, P], f32, tag="pp", name=f"warm_{i}")
                nc.tensor.matmul(ps, ones_st, ones_st, start=True, stop=True)

            # ---- prelude: first chains for pair (0, 0) ----
            qts = {0: qpool.tile([P, NFC, QG], bf16, tag="qT", name="qT_0")}
            kproj_chain(0, 0)
            qproj_chain(qts[0], 0, 0)

            # vproj ready slots matched to the xv transfer schedule
            VREADY = (9, 11, 12, 14)

            def push_vproj(hp2):
                for kt in range(NKT):
                    fillq.append(
                        (CHAIN_NS / 4.0, VREADY[kt // 4],
                         lambda kt=kt, hp2=hp2: vproj_unit(kt, hp2))
                    )

            push_vproj(0)

            ests_by_pair = {}
            attv_emitted = [0] * NPAIR   # qt count emitted per pair
            oT_by_g = {}

            def push_attv(t):
                g = t // NHP
                if g not in oT_by_g:
                    oT_by_g[g] = opool.tile(
                        [P, NFC, QG], bf16, tag="oT", name=f"oT_{g}"
                    )
                oT = oT_by_g[g]
                ests = ests_by_pair[t]
                g2, hp2 = divmod(t, NHP)
                if KNOB_OPBATCH:
                    for qt in range(NQG):
                        fillq.append(
                            (AV_NS, 0,
                             lambda t=t, qt=qt, ests=ests, oT=oT:
                                 attv_unit(t, qt, ests, oT))
                        )
                    if hp2 == NHP - 1:
                        for tt in range(NQG):
                            for eg in range(2):
                                fillq.append(
                                    (OPROJ_NS, 0,
                                     lambda g2=g2, tt=tt, eg=eg:
                                         outproj_unit(oT_by_g[g2], g2, tt, eg))
                                )
                else:
                    for qt in range(NQG):
                        def unit(t=t, qt=qt, ests=ests, oT=oT, g2=g2, hp2=hp2):
                            attv_unit(t, qt, ests, oT)
                            if hp2 == NHP - 1:
                                for eg in range(2):
                                    outproj_unit(oT_by_g[g2], g2, qt, eg)
                        fillq.append((AV_NS + (2 * OPROJ_NS if hp2 == NHP - 1 else 0), 0, unit))

            # ---- main loop: 16 pairs x 8 kt2 slots ----
            for t in range(NPAIR):
                g, hp = divmod(t, NHP)
                qT = qts[g]
                ha, hb = 2 * hp, 2 * hp + 1
                ests = {0: [], 1: []}
                ests_by_pair[t] = ests
                for kt2 in range(NKT // 2):
                    state["slot"] = t * 8 + kt2
                    # mandatory JIT splices
                    spent = 0.0
                    if g == 0 and kt2 in (1, 3, 5):
                        kproj_chain((kt2 + 1) // 2, hp)
                        spent += CHAIN_NS
                    if g == 0 and kt2 == 6 and hp + 1 < NFC:
                        kproj_chain(0, hp + 1)
                        spent += CHAIN_NS
                    if kt2 == 2 and t + 1 < NPAIR:
                        g1, fc1 = divmod(t + 1, NHP)
                        if g1 not in qts:
                            qts[g1] = qpool.tile(
                                [P, NFC, QG], bf16, tag="qT", name=f"qT_{g1}"
                            )
                        qproj_chain(qts[g1], g1, fc1)
                        spent += CHAIN_NS
                    # fill-queue drain for the rest of this slot's slack
                    drain(SLOT_NS - SCORE_NS - spent)
                    # scores for kt = 2*kt2, 2*kt2+1 (both heads) + exp, at
                    # high scheduler priority: the list scheduler then only
                    # runs fill work when the score/exp stream is blocked
                    with tc.high_priority(offset=500000 if KNOB_PRI_SCORE else 0):
                        sts = {
                            hi: st_pool.tile(
                                [P, 2, QG], f32, tag="st", name=f"st_{t}_{hi}_{kt2}"
                            )
                            for hi in range(2)
                        }
                        # h-major: sts[h0] completes one matmul earlier,
                        # so exp.h0 dispatches sooner at stall-recovery edges
                        for hi in range(2):
                            r0 = hi * DK
                            for kk in range(2):
                                kt = 2 * kt2 + kk
                                nc.tensor.matmul(
                                    sts[hi][:, kk, :],
                                    kT[r0 : r0 + DK, hp, ts(kt, P)],
                                    qT[r0 : r0 + DK, hp, :],
                                    start=True, stop=True, tile_position=(r0, 0),
                                )
                        for hi in range(2):
                            e = epool.tile(
                                [P, 2, QG], bf16, tag="est", name=f"est_{t}_{hi}_{kt2}"
                            )
                            ests[hi].append(e)
                            nc.scalar.activation(
                                out=e, in_=sts[hi], func=AF.Exp, scale=INV_SCALE
                            )
                # this pair's est tiles are complete -> queue its att@V;
                # stage the next head-pair's vproj ahead of the next attV
                push_attv(t)
                if t < NHP - 1:
                    push_vproj(t + 1)

            # ---- tail: drain everything left ----
            state["slot"] = 10**9
            drain(1e9)
            for i in range(16):
                ps = pp.tile([P, P], f32, tag="pp", name=f"cool_{i}")
                nc.tensor.matmul(ps, ones_st, ones_st, start=True, stop=True)

    nc.compile()
    return nc


def _get_nc(debug=False):
    if "nc" not in _CACHE:
        _CACHE["nc"] = _build()
    return _CACHE["nc"]


def _bf16(a):
    import ml_dtypes

    return np.ascontiguousarray(a, dtype=np.float32).astype(ml_dtypes.bfloat16)


def _make_in_maps(inputs):
    q = np.asarray(inputs["query"], dtype=np.float32)
    k = np.asarray(inputs["key"], dtype=np.float32)
    v = np.asarray(inputs["value"], dtype=np.float32)
    wq = np.asarray(inputs["wq"], dtype=np.float32)
    wk = np.asarray(inputs["wk"], dtype=np.float32)
    wv = np.asarray(inputs["wv"], dtype=np.float32)
    wo = np.asarray(inputs["wo"], dtype=np.float32)
    bq = np.asarray(inputs["bq"], dtype=np.float32)
    bk = np.asarray(inputs["bk"], dtype=np.float32)
    bv = np.asarray(inputs["bv"], dtype=np.float32)

    def _wsw(w):
        # [D, DC] -> [NFC, P, NDCH, P]: fc-major so one fc slice is a
        # single contiguous 256KB DMA
        return _bf16(
            np.ascontiguousarray(
                w.reshape(NDCH, P, NFC, P).transpose(2, 1, 0, 3)
            )
        )

    xT = [(_bf16(q[b].T), _bf16(k[b].T), _bf16(v[b].T)) for b in range(B)]
    in_maps = []
    for c in range(NCORES):
        b, g = divmod(c, 2)
        sl = slice(g * DC, (g + 1) * DC)
        in_maps.append(
            {
                "xqT": xT[b][0],
                "xkT": xT[b][1],
                "xvT": xT[b][2],
                "wq": _wsw(wq[:, sl]),
                "wk": _wsw(wk[:, sl]),
                "wv": _bf16(wv[:, sl]),
                "wo": _bf16(wo[sl, :]),
                "bq": np.ascontiguousarray(bq[sl]),
                "bk": np.ascontiguousarray(bk[sl]),
                "bv": np.ascontiguousarray(bv[sl]),
            }
        )
    return in_maps


def run(inputs, **kwargs):
    """Run the kernel; returns (full_output, BassKernelResults)."""
    from concourse.bass_utils import run_bass_kernel_spmd

    kwargs.pop("debug", None)
    nc = _get_nc()
    in_maps = _make_in_maps(inputs)
    res = run_bass_kernel_spmd(nc, in_maps, core_ids=list(range(NCORES)), **kwargs)
    bo = np.asarray(inputs["bo"], dtype=np.float32)
    final = np.empty((B, S, D), np.float32)
    for b in range(B):
        final[b] = (
            np.asarray(res.results[2 * b]["out"], dtype=np.float32)
            + np.asarray(res.results[2 * b + 1]["out"], dtype=np.float32)
            + bo
        )
    return final, res


def kernel(**inputs):
    return run(inputs)[0]
